# revision 18
# baseline (speedup 1.0000x reference)
"""Bahdanau attention kernel for Trainium2 (Bass/Tile), 8-core data-parallel.

Reference computation (per batch b):
    hp[o]      = sum_h hidden[b,h] * W_h[o,h] + b_h[o]            (+ b_c folded in)
    cp[s,o]    = sum_h enc[b,s,h] * W_c[o,h]
    energy     = tanh(hp + cp)                                     [S, H]
    scores[s]  = sum_o energy[s,o] * v[o]
    attn       = softmax(scores)                                   [S]
    context[h] = sum_s attn[s] * enc[b,s,h]                        [H]

Sharding: data-parallel over batch B=32 across 8 cores (4 batches/core);
H x H weights and v replicated.

Per-core kernel layout choices:
  - The big matmul (enc @ W_c^T, 17 GFLOP/core) contracts over h, so both
    operands need h on partitions.  enc arrives [s, h]; we transpose 128x128
    blocks on the PE (transpose mode) into encT [h, s] tiles.
  - cp tiles are produced [o=128p, s=512f] so the tanh bias (hp, per-o) is a
    per-partition ACT bias, and the v-dot is a K=128, M=1 matmul accumulating
    scores[1, 512] tiles in PSUM over the 8 o-chunks.
  - Matmuls run in float32r (1 cycle/row for free dim >= 256 vs 4 for fp32).
  - Softmax runs on a single partition: fused exp(x - max) + sum via one ACT
    instruction (accum_out), reciprocal + scale on DVE.
  - context matmul contracts over s, which is enc's natural partition layout:
    lhsT = attn column [128, 1], rhs = freshly re-DMAed enc rows.
"""

import os
import sys

sys.path.insert(0, "/opt/trn_rl_repo")

import numpy as np
from contextlib import ExitStack

import concourse.bass as bass
import concourse.bacc as bacc_mod
import concourse.mybir as mybir
import concourse.tile as tile
from concourse.masks import make_identity

B, S, H = 32, 2048, 1024
KREPS = int(os.environ.get("KREPS", "1"))  # repeat compute (timing calibration)
NCORES = 8
BL = B // NCORES  # 4 batches per core
P = 128
HC = H // P  # 8 contraction chunks
OC = H // P  # 8 output (o) chunks
SC = S // P  # 16 s chunks of 128
NST = S // 512  # 4 s tiles of 512

F32 = mybir.dt.float32
F32R = mybir.dt.float32r  # fast fp32 matmul mode
AF = mybir.ActivationFunctionType


def r(ap):
    """bitcast an fp32 AP to float32r for fast PE streaming."""
    return ap.bitcast(F32R)


def build_body(ctx, tc, hidden, enc, W_h, b_h, W_c, b_c, v, out_ctx, out_attn):
    nc = tc.nc

    const = ctx.enter_context(tc.tile_pool(name="const", bufs=1))
    wpool = ctx.enter_context(tc.tile_pool(name="wpool", bufs=1))  # WcT persistent
    wtmp = ctx.enter_context(tc.tile_pool(name="wtmp", bufs=4))  # natural W tiles
    bvp = ctx.enter_context(tc.tile_pool(name="bvp", bufs=1))  # bias/v/hidden rows
    natp = ctx.enter_context(tc.tile_pool(name="natp", bufs=8))  # "nat" tag  # enc natural stream
    encTp = ctx.enter_context(tc.tile_pool(name="encTp", bufs=1))  # encT + WhT slots
    epool = ctx.enter_context(tc.tile_pool(name="epool", bufs=3))  # energy tiles
    small = ctx.enter_context(tc.tile_pool(name="small", bufs=2))  # scores/attn/ctx

    ps_tr = ctx.enter_context(tc.tile_pool(name="ps_tr", bufs=2, space="PSUM"))
    ps_cp = ctx.enter_context(tc.tile_pool(name="ps_cp", bufs=2, space="PSUM"))
    ps_sc = ctx.enter_context(tc.tile_pool(name="ps_sc", bufs=2, space="PSUM"))
    ps_ms = ctx.enter_context(tc.tile_pool(name="ps_ms", bufs=2, space="PSUM"))

    # ---------------- phase 0: constants, weight transposes, hidden proj -----
    ident = const.tile([P, P], F32)
    make_identity(nc, ident)

    # b_h, b_c, v as rows of one [4, H] tile; hidden as [BL, H].
    # Small loads go on the ACT HWDGE ring (nc.scalar) so they overlap the
    # big W_c/W_h loads on the SP ring.
    bv = bvp.tile([4, H], F32, tag="bv")
    nc.vector.memset(bv, 0.0)
    nc.scalar.dma_start(bv[0:1, :], b_h[:].unsqueeze(0))
    nc.scalar.dma_start(bv[1:2, :], b_c[:].unsqueeze(0))
    nc.scalar.dma_start(bv[2:3, :], v[:].unsqueeze(0))
    hid4 = bvp.tile([BL, H], F32, tag="hid4")
    nc.scalar.dma_start(hid4, hidden[:, :])

    WcT = [wpool.tile([P, H], F32R, tag=f"wcT{h}", name=f"wcT{h}") for h in range(HC)]
    WhT = [encTp.tile([P, S], F32, tag=f"encT{h}", name=f"whT{h}") for h in range(HC)]

    def emit_w_transposes(W, WT):
        for og in range(2):  # groups of 4 o-row-chunks
            wnats = []
            for j in range(4):
                on = og * 4 + j
                t = wtmp.tile([P, H], F32, tag="wnat", name="wnat")
                nc.sync.dma_start(t, W[on * P : (on + 1) * P, :])
                wnats.append(t)
            for hc in range(HC):
                ps = ps_tr.tile([P, 512], F32)
                for j in range(4):
                    nc.tensor.transpose(
                        ps[:, j * P : (j + 1) * P],
                        wnats[j][:, hc * P : (hc + 1) * P],
                        ident,
                    )
                nc.vector.tensor_copy(WT[hc][:, og * 512 : (og + 1) * 512], ps)

    # W_c first: the PE can start on it as soon as the first 512 KB lands.
    emit_w_transposes(W_c, WcT)

    # transpose bv -> bvT [128, 8, 4]  (bvT[p, c, j] = bv[j, c*128+p])
    ps_bv = ps_ms.tile([P, 4 * HC], F32, tag="ms")
    for c in range(HC):
        nc.tensor.transpose(ps_bv[:, c * 4 : c * 4 + 4], bv[0:4, c * P : (c + 1) * P], ident[0:4, 0:4])
    bvT = const.tile([P, HC, 4], F32)
    nc.vector.tensor_copy(bvT, ps_bv)

    # transpose hidden -> hidT [128, 8, 4] (hidT[p, c, b] = hidden[b, c*128+p])
    ps_hd = ps_ms.tile([P, 4 * HC], F32, tag="ms")
    for c in range(HC):
        nc.tensor.transpose(ps_hd[:, c * 4 : c * 4 + 4], hid4[0:BL, c * P : (c + 1) * P], ident[0:BL, 0:BL])
    hidT = const.tile([P, HC, BL], F32)
    nc.vector.tensor_copy(hidT, ps_hd)

    # bias combine: bhc[p, c] = b_h + b_c at o = c*128+p
    bhc = const.tile([P, HC], F32)
    nc.vector.tensor_add(bhc, bvT[:, :, 0], bvT[:, :, 1])
    v_sb = const.tile([P, OC], F32R)
    nc.vector.tensor_copy(v_sb, bvT[:, :, 2])

    emit_w_transposes(W_h, WhT)

    # hidden projection: hp[o, b] = sum_h W_h[o,h] * hidden[b,h]  -> [128, 8, 4]
    ps_hp = ps_ms.tile([P, 4 * OC], F32, tag="ms")
    for on in range(OC):
        for hc in range(HC):
            nc.tensor.matmul(
                ps_hp[:, on * 4 : on * 4 + 4],
                WhT[hc][:, on * P : (on + 1) * P],
                hidT[:, hc, :],
                start=(hc == 0),
                stop=(hc == HC - 1),
            )
    hp_sb = const.tile([P, OC, BL], F32)
    for on in range(OC):
        nc.vector.tensor_scalar_add(
            hp_sb[:, on, :], ps_hp[:, on * 4 : on * 4 + 4], bhc[:, on : on + 1]
        )

    # ---------------- per-batch pipeline (software-pipelined) -----------------
    # Emission order per iteration b:
    #   enc transposes(b)  [PE]           <- overlaps softmax(b-1) on ACT/DVE
    #   attnT(b-1) + ctx(b-1)  [PE tail of previous batch]
    #   main(b): cp matmuls + tanh + delayed v-dot
    #   softmax(b)  [ACT/DVE]
    pending_tail = None

    def emit_tail(b, attn_sb):
        # -- transpose attn -> attnT [128, 16] --------------------------------
        ps_at = ps_ms.tile([P, SC], F32, tag="ms", name="ps_at")
        for sc in range(SC):
            nc.tensor.transpose(
                ps_at[:, sc : sc + 1],
                attn_sb[0:1, sc * P : (sc + 1) * P],
                ident[0:1, 0:1],
            )
        attnT = small.tile([P, SC], F32R, tag="attnT", name="attnT")
        nc.vector.tensor_copy(attnT, ps_at)

        # -- context: ctx[h] = sum_s attn[s] * enc[b, s, h] -------------------
        pctx0 = ps_ms.tile([1, 512], F32, tag="ms", name="pctx0")
        pctx1 = ps_ms.tile([1, 512], F32, tag="ms", name="pctx1")
        for sc in range(SC):
            nat2 = natp.tile([P, H], F32, tag="nat", name="nat2")
            nc.sync.dma_start(nat2, enc[b, sc * P : (sc + 1) * P, :])
            nat2r = natp.tile([P, H], F32R, tag="natr", name="natr", bufs=3)
            nc.gpsimd.tensor_copy(nat2r, nat2)
            nc.tensor.matmul(
                pctx0, attnT[:, sc : sc + 1], nat2r[:, 0:512],
                start=(sc == 0), stop=(sc == SC - 1),
            )
            nc.tensor.matmul(
                pctx1, attnT[:, sc : sc + 1], nat2r[:, 512:1024],
                start=(sc == 0), stop=(sc == SC - 1),
            )
        ctx_sb = small.tile([1, H], F32, tag="ctx", name="ctx_sb")
        nc.vector.tensor_copy(ctx_sb[0:1, 0:512], pctx0)
        nc.vector.tensor_copy(ctx_sb[0:1, 512:1024], pctx1)
        nc.sync.dma_start(out_ctx[b : b + 1, :], ctx_sb)

    for b_rep in range(KREPS * BL):
        b = b_rep % BL
        # -- load + transpose enc[b] into encT [h=128p x 8, s=2048f] ----------
        # fp32r transpose mode: 1.5 cycles/row instead of 2 for fp32.
        encT = [encTp.tile([P, S], F32R, tag=f"encT{h}", name=f"encT{h}") for h in range(HC)]
        for g in range(4):  # groups of 4 s-chunks
            nats = []
            for j in range(4):
                sc = g * 4 + j
                t = natp.tile([P, H], F32, tag="nat", name="nat")
                nc.sync.dma_start(t, enc[b, sc * P : (sc + 1) * P, :])
                nats.append(t)
            for hc in range(HC):
                ps = ps_tr.tile([P, 512], F32)
                for j in range(4):
                    nc.tensor.transpose(
                        ps[:, j * P : (j + 1) * P],
                        nats[j][:, hc * P : (hc + 1) * P],
                        ident,
                    )
                nc.vector.tensor_copy(encT[hc][:, g * 512 : (g + 1) * 512], ps)

        # -- previous batch's attn transpose + context matmul -----------------
        if pending_tail is not None:
            pending_tail()
            pending_tail = None

        # -- main loop: cp matmul + tanh + delayed v-dot ----------------------
        sc_sb = small.tile([1, S], F32, tag="sc")
        sc_banks = [None] * NST
        pending = None  # (oc, st, E-tile) whose v-dot is not yet emitted

        def emit_vdot(oc, st, E):
            if oc == 0:
                sc_banks[st] = ps_sc.tile([1, 512], F32, tag="scps", name="scps")
            nc.tensor.matmul(
                sc_banks[st],
                r(v_sb[:, oc : oc + 1]),
                r(E),
                start=(oc == 0),
                stop=(oc == OC - 1),
            )
            if oc == OC - 1:
                nc.vector.tensor_copy(sc_sb[0:1, st * 512 : (st + 1) * 512], sc_banks[st])

        for st in range(NST):
            for oc in range(OC):
                pcp = ps_cp.tile([P, 512], F32)
                for hc in range(HC):
                    nc.tensor.matmul(
                        pcp,
                        r(WcT[hc][:, oc * P : (oc + 1) * P]),
                        r(encT[hc][:, st * 512 : (st + 1) * 512]),
                        start=(hc == 0),
                        stop=(hc == HC - 1),
                    )
                E = epool.tile([P, 512], F32R, tag="E")
                nc.scalar.activation(
                    E, pcp, AF.Tanh, bias=hp_sb[:, oc, b : b + 1], scale=1.0
                )
                if pending is not None:
                    emit_vdot(*pending)
                pending = (oc, st, E)
        emit_vdot(*pending)
        pending = None

        # -- softmax over the 2048 scores (single partition) ------------------
        negmax = small.tile([1, 1], F32, tag="negmax")
        nc.vector.tensor_reduce(
            negmax, sc_sb[0:1, :], axis=mybir.AxisListType.X,
            op=mybir.AluOpType.max, negate=True,
        )
        ssum = small.tile([1, 1], F32, tag="ssum")
        nc.scalar.activation(
            sc_sb[0:1, :], sc_sb[0:1, :], AF.Exp, bias=negmax[0:1, 0:1], scale=1.0,
            accum_out=ssum,
        )
        rinv = small.tile([1, 1], F32, tag="rinv")
        nc.vector.reciprocal(rinv, ssum)
        attn_sb = sc_sb
        nc.vector.tensor_scalar_mul(attn_sb[0:1, :], attn_sb[0:1, :], rinv[0:1, 0:1])
        nc.sync.dma_start(out_attn[b : b + 1, :], attn_sb[0:1, :])

        pending_tail = (lambda bb=b, asb=attn_sb: emit_tail(bb, asb))

    pending_tail()
    pending_tail = None


def build_bass():
    nc = bacc_mod.Bacc(None, target_bir_lowering=False)
    hidden = nc.dram_tensor("hidden", [BL, H], F32, kind="ExternalInput")
    enc = nc.dram_tensor("encoder_outputs", [BL, S, H], F32, kind="ExternalInput")
    W_h = nc.dram_tensor("W_h", [H, H], F32, kind="ExternalInput")
    b_h = nc.dram_tensor("b_h", [H], F32, kind="ExternalInput")
    W_c = nc.dram_tensor("W_c", [H, H], F32, kind="ExternalInput")
    b_c = nc.dram_tensor("b_c", [H], F32, kind="ExternalInput")
    v = nc.dram_tensor("v", [H], F32, kind="ExternalInput")
    out_ctx = nc.dram_tensor("context", [BL, H], F32, kind="ExternalOutput")
    out_attn = nc.dram_tensor("attn", [BL, S], F32, kind="ExternalOutput")

    with tile.TileContext(nc) as tc:
        with ExitStack() as ctx:
            build_body(ctx, tc, hidden, enc, W_h, b_h, W_c, b_c, v, out_ctx, out_attn)
    nc.compile()
    return nc


_NC_CACHE = None


def get_nc():
    global _NC_CACHE
    if _NC_CACHE is None:
        _NC_CACHE = build_bass()
    return _NC_CACHE


def kernel(**inputs):
    from concourse.bass_utils import run_bass_kernel_spmd

    hidden = np.ascontiguousarray(np.asarray(inputs["hidden"], dtype=np.float32))
    enc = np.ascontiguousarray(np.asarray(inputs["encoder_outputs"], dtype=np.float32))
    W_h = np.ascontiguousarray(np.asarray(inputs["W_h"], dtype=np.float32))
    b_h = np.ascontiguousarray(np.asarray(inputs["b_h"], dtype=np.float32))
    W_c = np.ascontiguousarray(np.asarray(inputs["W_c"], dtype=np.float32))
    b_c = np.ascontiguousarray(np.asarray(inputs["b_c"], dtype=np.float32))
    v = np.ascontiguousarray(np.asarray(inputs["v"], dtype=np.float32))

    nc = get_nc()
    in_maps = []
    for i in range(NCORES):
        sl = slice(i * BL, (i + 1) * BL)
        in_maps.append(
            {
                "hidden": np.ascontiguousarray(hidden[sl]),
                "encoder_outputs": np.ascontiguousarray(enc[sl]),
                "W_h": W_h,
                "b_h": b_h,
                "W_c": W_c,
                "b_c": b_c,
                "v": v,
            }
        )

    res = run_bass_kernel_spmd(nc, in_maps, core_ids=list(range(NCORES)))
    context = np.concatenate([res.results[i]["context"] for i in range(NCORES)], axis=0)
    attn = np.concatenate([res.results[i]["attn"] for i in range(NCORES)], axis=0)
    return (context, attn)


if __name__ == "__main__":
    nc = build_bass()
    print("built ok; instructions:", len(nc.inst_map))


# revision 23
# speedup vs baseline: 189.4712x; 189.4712x over previous
"""Bahdanau attention kernel for Trainium2 (Bass/Tile), 8-core data-parallel.

Reference computation (per batch b):
    hp[o]      = sum_h hidden[b,h] * W_h[o,h] + b_h[o]            (+ b_c folded in)
    cp[s,o]    = sum_h enc[b,s,h] * W_c[o,h]
    energy     = tanh(hp + cp)                                     [S, H]
    scores[s]  = sum_o energy[s,o] * v[o]
    attn       = softmax(scores)                                   [S]
    context[h] = sum_s attn[s] * enc[b,s,h]                        [H]

Sharding: data-parallel over batch B=32 across 8 cores (4 batches/core);
H x H weights and v replicated.

Per-core kernel layout choices:
  - The big matmul (enc @ W_c^T, 17 GFLOP/core) contracts over h, so both
    operands need h on partitions.  enc arrives [s, h]; we transpose 128x128
    blocks on the PE (transpose mode) into encT [h, s] tiles.
  - cp tiles are produced [o=128p, s=512f] so the tanh bias (hp, per-o) is a
    per-partition ACT bias, and the v-dot is a K=128, M=1 matmul accumulating
    scores[1, 512] tiles in PSUM over the 8 o-chunks.
  - Matmuls run in float32r (1 cycle/row for free dim >= 256 vs 4 for fp32).
  - Softmax runs on a single partition: fused exp(x - max) + sum via one ACT
    instruction (accum_out), reciprocal + scale on DVE.
  - context matmul contracts over s, which is enc's natural partition layout:
    lhsT = attn column [128, 1], rhs = freshly re-DMAed enc rows.
"""

import os
import sys

sys.path.insert(0, "/opt/trn_rl_repo")

import numpy as np
from contextlib import ExitStack

import concourse.bass as bass
import concourse.bacc as bacc_mod
import concourse.mybir as mybir
import concourse.tile as tile
from concourse.masks import make_identity

B, S, H = 32, 2048, 1024
KREPS = int(os.environ.get("KREPS", "1"))  # repeat compute (timing calibration)
NCORES = 8
BL = B // NCORES  # 4 batches per core
P = 128
HC = H // P  # 8 contraction chunks
OC = H // P  # 8 output (o) chunks
SC = S // P  # 16 s chunks of 128
NST = S // 512  # 4 s tiles of 512

F32 = mybir.dt.float32
F32R = mybir.dt.float32r  # fast fp32 matmul mode
AF = mybir.ActivationFunctionType


def r(ap):
    """bitcast an fp32 AP to float32r for fast PE streaming."""
    return ap.bitcast(F32R)


def build_body(ctx, tc, hidden, enc, W_h, b_h, W_c, b_c, v, out_ctx, out_attn):
    nc = tc.nc

    const = ctx.enter_context(tc.tile_pool(name="const", bufs=1))
    wpool = ctx.enter_context(tc.tile_pool(name="wpool", bufs=1))  # WcT persistent
    wtmp = ctx.enter_context(tc.tile_pool(name="wtmp", bufs=4))  # natural W tiles
    bvp = ctx.enter_context(tc.tile_pool(name="bvp", bufs=1))  # bias/v/hidden rows
    natp = ctx.enter_context(tc.tile_pool(name="natp", bufs=8))  # "nat" tag  # enc natural stream
    encTp = ctx.enter_context(tc.tile_pool(name="encTp", bufs=1))  # encT + WhT slots
    epool = ctx.enter_context(tc.tile_pool(name="epool", bufs=3))  # energy tiles
    small = ctx.enter_context(tc.tile_pool(name="small", bufs=2))  # scores/attn/ctx

    ps_tr = ctx.enter_context(tc.tile_pool(name="ps_tr", bufs=2, space="PSUM"))
    ps_cp = ctx.enter_context(tc.tile_pool(name="ps_cp", bufs=2, space="PSUM"))
    ps_sc = ctx.enter_context(tc.tile_pool(name="ps_sc", bufs=2, space="PSUM"))
    ps_ms = ctx.enter_context(tc.tile_pool(name="ps_ms", bufs=2, space="PSUM"))

    # ---------------- phase 0: constants, weight transposes, hidden proj -----
    ident = const.tile([P, P], F32)
    make_identity(nc, ident)

    # b_h, b_c, v as rows of one [4, H] tile; hidden as [BL, H].
    # Small loads go on the ACT HWDGE ring (nc.scalar) so they overlap the
    # big W_c/W_h loads on the SP ring.
    bv = bvp.tile([4, H], F32, tag="bv")
    nc.vector.memset(bv, 0.0)
    nc.scalar.dma_start(bv[0:1, :], b_h[:].unsqueeze(0))
    nc.scalar.dma_start(bv[1:2, :], b_c[:].unsqueeze(0))
    nc.scalar.dma_start(bv[2:3, :], v[:].unsqueeze(0))
    hid4 = bvp.tile([BL, H], F32, tag="hid4")
    nc.scalar.dma_start(hid4, hidden[:, :])

    WcT = [wpool.tile([P, H], F32R, tag=f"wcT{h}", name=f"wcT{h}") for h in range(HC)]
    WhT = [encTp.tile([P, S], F32, tag=f"encT{h}", name=f"whT{h}") for h in range(HC)]

    def emit_w_transposes(W, WT):
        for og in range(2):  # groups of 4 o-row-chunks
            wnats = []
            for j in range(4):
                on = og * 4 + j
                t = wtmp.tile([P, H], F32, tag="wnat", name="wnat")
                nc.sync.dma_start(t, W[on * P : (on + 1) * P, :])
                wnats.append(t)
            for hc in range(HC):
                ps = ps_tr.tile([P, 512], F32)
                for j in range(4):
                    nc.tensor.transpose(
                        ps[:, j * P : (j + 1) * P],
                        wnats[j][:, hc * P : (hc + 1) * P],
                        ident,
                    )
                nc.vector.tensor_copy(WT[hc][:, og * 512 : (og + 1) * 512], ps)

    # W_c first: the PE can start on it as soon as the first 512 KB lands.
    emit_w_transposes(W_c, WcT)

    # transpose bv -> bvT [128, 8, 4]  (bvT[p, c, j] = bv[j, c*128+p])
    ps_bv = ps_ms.tile([P, 4 * HC], F32, tag="ms")
    for c in range(HC):
        nc.tensor.transpose(ps_bv[:, c * 4 : c * 4 + 4], bv[0:4, c * P : (c + 1) * P], ident[0:4, 0:4])
    bvT = const.tile([P, HC, 4], F32)
    nc.vector.tensor_copy(bvT, ps_bv)

    # transpose hidden -> hidT [128, 8, 4] (hidT[p, c, b] = hidden[b, c*128+p])
    ps_hd = ps_ms.tile([P, 4 * HC], F32, tag="ms")
    for c in range(HC):
        nc.tensor.transpose(ps_hd[:, c * 4 : c * 4 + 4], hid4[0:BL, c * P : (c + 1) * P], ident[0:BL, 0:BL])
    hidT = const.tile([P, HC, BL], F32)
    nc.vector.tensor_copy(hidT, ps_hd)

    # bias combine: bhc[p, c] = b_h + b_c at o = c*128+p
    bhc = const.tile([P, HC], F32)
    nc.vector.tensor_add(bhc, bvT[:, :, 0], bvT[:, :, 1])
    v_sb = const.tile([P, OC], F32R)
    nc.vector.tensor_copy(v_sb, bvT[:, :, 2])

    emit_w_transposes(W_h, WhT)

    # hidden projection: hp[o, b] = sum_h W_h[o,h] * hidden[b,h]  -> [128, 8, 4]
    ps_hp = ps_ms.tile([P, 4 * OC], F32, tag="ms")
    for on in range(OC):
        for hc in range(HC):
            nc.tensor.matmul(
                ps_hp[:, on * 4 : on * 4 + 4],
                WhT[hc][:, on * P : (on + 1) * P],
                hidT[:, hc, :],
                start=(hc == 0),
                stop=(hc == HC - 1),
            )
    hp_sb = const.tile([P, OC, BL], F32)
    for on in range(OC):
        nc.vector.tensor_scalar_add(
            hp_sb[:, on, :], ps_hp[:, on * 4 : on * 4 + 4], bhc[:, on : on + 1]
        )

    # ---------------- per-batch pipeline (software-pipelined) -----------------
    # Emission order per iteration b:
    #   enc transposes(b)  [PE]           <- overlaps softmax(b-1) on ACT/DVE
    #   attnT(b-1) + ctx(b-1)  [PE tail of previous batch]
    #   main(b): cp matmuls + tanh + delayed v-dot
    #   softmax(b)  [ACT/DVE]
    pending_tail = None

    def emit_tail(b, attn_sb):
        # -- transpose attn -> attnT [128, 16] --------------------------------
        ps_at = ps_ms.tile([P, SC], F32, tag="ms", name="ps_at")
        for sc in range(SC):
            nc.tensor.transpose(
                ps_at[:, sc : sc + 1],
                attn_sb[0:1, sc * P : (sc + 1) * P],
                ident[0:1, 0:1],
            )
        attnT = small.tile([P, SC], F32R, tag="attnT", name="attnT")
        nc.vector.tensor_copy(attnT, ps_at)

        # -- context: ctx[h] = sum_s attn[s] * enc[b, s, h] -------------------
        pctx0 = ps_ms.tile([1, 512], F32, tag="ms", name="pctx0")
        pctx1 = ps_ms.tile([1, 512], F32, tag="ms", name="pctx1")
        for sc in range(SC):
            nat2 = natp.tile([P, H], F32, tag="nat", name="nat2")
            nc.sync.dma_start(nat2, enc[b, sc * P : (sc + 1) * P, :])
            nat2r = natp.tile([P, H], F32R, tag="natr", name="natr", bufs=4)
            nc.gpsimd.tensor_copy(nat2r, nat2)
            nc.tensor.matmul(
                pctx0, attnT[:, sc : sc + 1], nat2r[:, 0:512],
                start=(sc == 0), stop=(sc == SC - 1),
            )
            nc.tensor.matmul(
                pctx1, attnT[:, sc : sc + 1], nat2r[:, 512:1024],
                start=(sc == 0), stop=(sc == SC - 1),
            )
        ctx_sb = small.tile([1, H], F32, tag="ctx", name="ctx_sb")
        nc.vector.tensor_copy(ctx_sb[0:1, 0:512], pctx0)
        nc.vector.tensor_copy(ctx_sb[0:1, 512:1024], pctx1)
        nc.sync.dma_start(out_ctx[b : b + 1, :], ctx_sb)

    for b_rep in range(KREPS * BL):
        b = b_rep % BL
        # -- load + transpose enc[b] into encT [h=128p x 8, s=2048f] ----------
        # fp32r transpose mode: 1.5 cycles/row instead of 2 for fp32.
        encT = [encTp.tile([P, S], F32R, tag=f"encT{h}", name=f"encT{h}") for h in range(HC)]
        for g in range(4):  # groups of 4 s-chunks
            nats = []
            for j in range(4):
                sc = g * 4 + j
                t = natp.tile([P, H], F32, tag="nat", name="nat")
                nc.sync.dma_start(t, enc[b, sc * P : (sc + 1) * P, :])
                nats.append(t)
            for hc in range(HC):
                ps = ps_tr.tile([P, 512], F32)
                for j in range(4):
                    nc.tensor.transpose(
                        ps[:, j * P : (j + 1) * P],
                        nats[j][:, hc * P : (hc + 1) * P],
                        ident,
                    )
                nc.vector.tensor_copy(encT[hc][:, g * 512 : (g + 1) * 512], ps)

        # -- previous batch's attn transpose + context matmul -----------------
        if pending_tail is not None:
            pending_tail()
            pending_tail = None

        # -- main loop: cp matmul + tanh + delayed v-dot ----------------------
        sc_sb = small.tile([1, S], F32, tag="sc")
        sc_banks = [None] * NST
        pending = None  # (oc, st, E-tile) whose v-dot is not yet emitted

        def emit_vdot(oc, st, E):
            if oc == 0:
                sc_banks[st] = ps_sc.tile([1, 512], F32, tag="scps", name="scps")
            nc.tensor.matmul(
                sc_banks[st],
                r(v_sb[:, oc : oc + 1]),
                r(E),
                start=(oc == 0),
                stop=(oc == OC - 1),
            )
            if oc == OC - 1:
                nc.vector.tensor_copy(sc_sb[0:1, st * 512 : (st + 1) * 512], sc_banks[st])

        for st in range(NST):
            for oc in range(OC):
                pcp = ps_cp.tile([P, 512], F32)
                for hc in range(HC):
                    nc.tensor.matmul(
                        pcp,
                        r(WcT[hc][:, oc * P : (oc + 1) * P]),
                        r(encT[hc][:, st * 512 : (st + 1) * 512]),
                        start=(hc == 0),
                        stop=(hc == HC - 1),
                    )
                E = epool.tile([P, 512], F32R, tag="E")
                nc.scalar.activation(
                    E, pcp, AF.Tanh, bias=hp_sb[:, oc, b : b + 1], scale=1.0
                )
                if pending is not None:
                    emit_vdot(*pending)
                pending = (oc, st, E)
        emit_vdot(*pending)
        pending = None

        # -- softmax over the 2048 scores (single partition) ------------------
        negmax = small.tile([1, 1], F32, tag="negmax")
        nc.vector.tensor_reduce(
            negmax, sc_sb[0:1, :], axis=mybir.AxisListType.X,
            op=mybir.AluOpType.max, negate=True,
        )
        ssum = small.tile([1, 1], F32, tag="ssum")
        nc.scalar.activation(
            sc_sb[0:1, :], sc_sb[0:1, :], AF.Exp, bias=negmax[0:1, 0:1], scale=1.0,
            accum_out=ssum,
        )
        rinv = small.tile([1, 1], F32, tag="rinv")
        nc.vector.reciprocal(rinv, ssum)
        attn_sb = sc_sb
        nc.vector.tensor_scalar_mul(attn_sb[0:1, :], attn_sb[0:1, :], rinv[0:1, 0:1])
        nc.sync.dma_start(out_attn[b : b + 1, :], attn_sb[0:1, :])

        pending_tail = (lambda bb=b, asb=attn_sb: emit_tail(bb, asb))

    pending_tail()
    pending_tail = None


def build_bass():
    nc = bacc_mod.Bacc(None, target_bir_lowering=False)
    hidden = nc.dram_tensor("hidden", [BL, H], F32, kind="ExternalInput")
    enc = nc.dram_tensor("encoder_outputs", [BL, S, H], F32, kind="ExternalInput")
    W_h = nc.dram_tensor("W_h", [H, H], F32, kind="ExternalInput")
    b_h = nc.dram_tensor("b_h", [H], F32, kind="ExternalInput")
    W_c = nc.dram_tensor("W_c", [H, H], F32, kind="ExternalInput")
    b_c = nc.dram_tensor("b_c", [H], F32, kind="ExternalInput")
    v = nc.dram_tensor("v", [H], F32, kind="ExternalInput")
    out_ctx = nc.dram_tensor("context", [BL, H], F32, kind="ExternalOutput")
    out_attn = nc.dram_tensor("attn", [BL, S], F32, kind="ExternalOutput")

    with tile.TileContext(nc) as tc:
        with ExitStack() as ctx:
            build_body(ctx, tc, hidden, enc, W_h, b_h, W_c, b_c, v, out_ctx, out_attn)
    nc.compile()
    return nc


_NC_CACHE = None


def get_nc():
    global _NC_CACHE
    if _NC_CACHE is None:
        _NC_CACHE = build_bass()
    return _NC_CACHE


def kernel(**inputs):
    from concourse.bass_utils import run_bass_kernel_spmd

    hidden = np.ascontiguousarray(np.asarray(inputs["hidden"], dtype=np.float32))
    enc = np.ascontiguousarray(np.asarray(inputs["encoder_outputs"], dtype=np.float32))
    W_h = np.ascontiguousarray(np.asarray(inputs["W_h"], dtype=np.float32))
    b_h = np.ascontiguousarray(np.asarray(inputs["b_h"], dtype=np.float32))
    W_c = np.ascontiguousarray(np.asarray(inputs["W_c"], dtype=np.float32))
    b_c = np.ascontiguousarray(np.asarray(inputs["b_c"], dtype=np.float32))
    v = np.ascontiguousarray(np.asarray(inputs["v"], dtype=np.float32))

    nc = get_nc()
    in_maps = []
    for i in range(NCORES):
        sl = slice(i * BL, (i + 1) * BL)
        in_maps.append(
            {
                "hidden": np.ascontiguousarray(hidden[sl]),
                "encoder_outputs": np.ascontiguousarray(enc[sl]),
                "W_h": W_h,
                "b_h": b_h,
                "W_c": W_c,
                "b_c": b_c,
                "v": v,
            }
        )

    res = run_bass_kernel_spmd(nc, in_maps, core_ids=list(range(NCORES)))
    context = np.concatenate([res.results[i]["context"] for i in range(NCORES)], axis=0)
    attn = np.concatenate([res.results[i]["attn"] for i in range(NCORES)], axis=0)
    return (context, attn)


if __name__ == "__main__":
    nc = build_bass()
    print("built ok; instructions:", len(nc.inst_map))
